# revision 5
# baseline (speedup 1.0000x reference)
"""Multi-head attention on 8 TRN2 NeuronCores (data/head-parallel).

Problem: B=4 H=16 S=2048 D=64 fp32 attention, out = softmax(Q K^T / sqrt(D)) V.
B*H = 64 (batch, head) pairs are sharded 8-per-core; each core runs the same
NEFF over its own 8 heads, no collectives.

v4 design (v1 ~287us, v2 ~257us, v3 ~253us):
  - QK^T uses 2-way PE row tiling (64x128 mode, tiles T0/T8): k-tile 2j's
    K^T sits in array rows 0:64, k-tile 2j+1's in rows 64:128, Q^T is
    host-duplicated into both SBUF partition halves, and the two score
    matmuls run concurrently (~2x QK). One stationary load covers 1024 q
    columns (2 sub-blocks) to amortize the LDWEIGHTS that cannot hide
    behind a same-row-group matmul.
  - exp is spread over THREE engines. ACT does exact exp for most k-tile
    pairs. The rest use a one-instruction Schraudolph exponential —
    i16 = round(score * 128*log2(e)/sqrt(D) + 16248.5) is the bit pattern
    of bf16(exp(score/sqrt(D))) to ~1.8% rms — either directly on DVE
    (reads PSUM), or on GpSimd (cannot touch PSUM) fed by a DVE
    fp32->bf16 bounce copy. Softmax's scale invariance cancels the
    common-mode part of the approximation error.
  - V gets a ones column appended, so PV's PSUM accumulator holds the
    unnormalized output transpose [65, q] with softmax denominators in
    row 64. It is copied to SBUF and DMAd out as-is; division and the
    [d, q] -> [q, d] transpose happen on the host, freeing PE transposes
    and the DVE reciprocal/scale chain.
  - All steady-state DMA issue lives on the Sync queue (GpSimd now does
    exp work); kt/vp are host-packed so every load is contiguous per
    partition (2-4KB descriptors).
"""

import math
from contextlib import ExitStack

import ml_dtypes
import numpy as np

import concourse.bass as bass
import concourse.bacc as bacc
import concourse.tile as tile
import concourse.mybir as mybir
from concourse.bass_utils import run_bass_kernel_spmd

B, H, S, D = 4, 16, 2048, 64
N_CORES = 8
HPC = B * H // N_CORES     # heads per core
NPAIR = 8                  # k-tile pairs (16 k-tiles of 128)
QB = 512                   # q sub-block (one PSUM bank of scores per k-tile)
SB = 1024                  # q super-block (one stationary load per k-tile)
NSB = S // SB
DT = mybir.dt

# Schraudolph-in-bf16-bit-space constants (exp(score/8) ~= bf16 bits of
# round(score * A + Bc) as int16). Bc calibrated for round-to-nearest.
EXP_A = 128.0 * 1.4426950408889634 / 8.0
EXP_B = 16248.5
# exp engine per (pair, sub-block) slot: 'a' = ACT exact, 'v' = DVE
# Schraudolph from PSUM, 'g' = GpSimd Schraudolph from a DVE bf16 bounce.
ENGINE_MAP = {
    0: "aa", 1: "aa", 2: "vv", 3: "aa",
    4: "gg", 5: "aa", 6: "gg", 7: "av",
}

_BUILT = {}


class _Bacc(bacc.Bacc):
    """Bacc with the move-matmul-waits-to-ldweights pass disabled: keeping
    waits on the matmul (not its LDWEIGHTS) lets the PE queue pull weight
    loads ahead of in-flight matmuls, hiding the ~70ns LDW cost."""

    def move_matmul_waits_to_ldweights(self):
        pass


def _head(nc, pools, scale, qt_d, kt_d, vp_d, o_d, h):
    (stage, epool, spool, ps_st, ps_ot) = pools

    # ---- loads (bf16, pre-transposed + packed on host) -------------------
    # qt rows 0:64 / 64:128 both hold Q^T. kt packs k-tile pairs: rows 0:64
    # = K^T of tile 2j, rows 64:128 = K^T of tile 2j+1, pair-major in the
    # free dim. vp is host-packed partition-major: vp[p, t*128:] = V' row
    # t*128+p. All three are contiguous per partition in HBM.
    qt = stage.tile([128, S], DT.bfloat16, tag="qt")
    kt = stage.tile([128, NPAIR * 128], DT.bfloat16, tag="kt")
    vp = stage.tile([128, 2 * NPAIR * 128], DT.bfloat16, tag="vp")
    if h == 0:
        # first matmul group needs kt + qt's first super-block; issue those
        # on otherwise-idle queues to cut the cold prologue.
        nc.scalar.dma_start(out=kt, in_=kt_d[h])
        nc.scalar.dma_start(out=qt[:, 0:SB], in_=qt_d[h][:, 0:SB])
        nc.sync.dma_start(out=qt[:, SB:], in_=qt_d[h][:, SB:])
        nc.gpsimd.dma_start(out=vp, in_=vp_d[h])
    else:
        nc.sync.dma_start(out=kt, in_=kt_d[h])
        for j in range(2):
            half = slice(j * (S // 2), (j + 1) * (S // 2))
            nc.sync.dma_start(out=qt[:, half], in_=qt_d[h][:, half])
        nc.sync.dma_start(out=vp, in_=vp_d[h])

    # ---- attention per q super-block -------------------------------------
    for c in range(NSB):
        q0 = c * SB
        ets = []
        for j in range(NPAIR):
            # row-tiled score pair: T0 (rows 0:64) does k-tile 2j, T8
            # (rows 64:128) does k-tile 2j+1, concurrently. One stationary
            # load covers both 512-wide sub-blocks; the sub-blocks land in
            # adjacent PSUM banks of per-sub-block [128, 1024] tiles.
            st0 = ps_st.tile([128, 2 * QB], DT.float32, tag="st")
            st1 = ps_st.tile([128, 2 * QB], DT.float32, tag="st")
            for s, st in enumerate((st0, st1)):
                nc.tensor.matmul(
                    st[:, 0:QB],
                    lhsT=kt[0:64, j * 128 : (j + 1) * 128],
                    rhs=qt[0:64, q0 + s * QB : q0 + (s + 1) * QB],
                    start=True,
                    stop=True,
                )
            for s, st in enumerate((st0, st1)):
                nc.tensor.matmul(
                    st[:, QB : 2 * QB],
                    lhsT=kt[64:128, j * 128 : (j + 1) * 128],
                    rhs=qt[64:128, q0 + s * QB : q0 + (s + 1) * QB],
                    start=True,
                    stop=True,
                )
            for s, st in enumerate((st0, st1)):
                et = epool.tile([128, 2 * QB], DT.bfloat16, tag=f"et{j}_{s}")
                eng = ENGINE_MAP[j][s]
                if eng == "a":
                    nc.scalar.activation(
                        out=et, in_=st,
                        func=mybir.ActivationFunctionType.Exp, scale=scale,
                    )
                elif eng == "v":
                    nc.vector.tensor_scalar(
                        et.bitcast(DT.int16), st, EXP_A, EXP_B,
                        mybir.AluOpType.mult, mybir.AluOpType.add,
                    )
                else:
                    scb = epool.tile([128, 2 * QB], DT.bfloat16,
                                     tag=f"scb{j}_{s}")
                    nc.vector.tensor_copy(out=scb, in_=st)
                    nc.gpsimd.tensor_scalar(
                        et.bitcast(DT.int16), scb, EXP_A, EXP_B,
                        mybir.AluOpType.mult, mybir.AluOpType.add,
                    )
                ets.append(et)

        ot = ps_ot.tile([128, SB], DT.float32, tag="ot")
        for s in range(2):
            for j in range(NPAIR):
                for half in range(2):
                    t = 2 * j + half
                    nc.tensor.matmul(
                        ot[:, s * QB : (s + 1) * QB],
                        lhsT=vp[:, t * 128 : (t + 1) * 128],
                        rhs=ets[2 * j + s][:, half * QB : (half + 1) * QB],
                        start=(t == 0),
                        stop=(t == 2 * NPAIR - 1),
                    )
        # unnormalized out^T: rows 0:64 numerator, row 64 denominator.
        # Division + transpose happen on the host.
        ots = spool.tile([D + 1, SB], DT.float32, tag="ots")
        nc.vector.tensor_copy(out=ots, in_=ot[0 : D + 1, :])
        nc.sync.dma_start(out=o_d[h][:, q0 : q0 + SB], in_=ots)


def build_graph(scale: float, heads: int = HPC):
    nc = _Bacc("TRN2", target_bir_lowering=False, debug=False,
               num_devices=N_CORES)
    qt_d = nc.dram_tensor("QT", [heads, 128, S], DT.bfloat16,
                          kind="ExternalInput").ap()
    kt_d = nc.dram_tensor("KT", [heads, 128, NPAIR * 128], DT.bfloat16,
                          kind="ExternalInput").ap()
    vp_d = nc.dram_tensor("VP", [heads, 128, 2 * NPAIR * 128], DT.bfloat16,
                          kind="ExternalInput").ap()
    o_d = nc.dram_tensor("out", [heads, D + 1, S], DT.float32,
                         kind="ExternalOutput").ap()

    with tile.TileContext(nc) as tc, ExitStack() as ctx:
        stage = ctx.enter_context(tc.tile_pool(name="stage", bufs=2))
        epool = ctx.enter_context(tc.tile_pool(name="epool", bufs=2))
        spool = ctx.enter_context(tc.tile_pool(name="spool", bufs=3))
        ps_st = ctx.enter_context(tc.tile_pool(name="ps_st", bufs=3, space="PSUM"))
        ps_ot = ctx.enter_context(tc.tile_pool(name="ps_ot", bufs=1, space="PSUM"))

        pools = (stage, epool, spool, ps_st, ps_ot)
        for h in range(heads):
            _head(nc, pools, scale, qt_d, kt_d, vp_d, o_d, h)

    nc.compile()
    return nc


def _get_nc(scale: float):
    key = round(float(scale), 9)
    if key not in _BUILT:
        _BUILT[key] = build_graph(float(scale))
    return _BUILT[key]


def shard_inputs(Q, K, V):
    """Host-side prep: shard heads across cores; build qt (Q^T duplicated
    into both 64-row halves), kt (k-tile pairs packed for row tiling), vp
    (V plus a ones column, partition-major so DMA is contiguous)."""
    bf16 = ml_dtypes.bfloat16
    qs = np.asarray(Q, dtype=np.float32).reshape(B * H, S, D)
    ks = np.asarray(K, dtype=np.float32).reshape(B * H, S, D)
    vs = np.asarray(V, dtype=np.float32).reshape(B * H, S, D)
    qtT = qs.transpose(0, 2, 1).astype(bf16)          # [BH, D, S]
    qt = np.concatenate([qtT, qtT], axis=1)           # [BH, 128, S]
    ktT = ks.transpose(0, 2, 1).astype(bf16)          # [BH, D, S]
    # pairs: rows 0:64 <- k-tile 2j, rows 64:128 <- k-tile 2j+1
    ktv = ktT.reshape(B * H, D, NPAIR, 2, 128)
    kt = np.empty((B * H, 128, NPAIR, 128), dtype=bf16)
    kt[:, :D] = ktv[:, :, :, 0, :]
    kt[:, D:] = ktv[:, :, :, 1, :]
    kt = kt.reshape(B * H, 128, NPAIR * 128)
    # vp partition-major: vp[bh, p, t*128 + e] = V'[bh, t*128 + p, e]
    vpb = np.zeros((B * H, S, 128), dtype=bf16)
    vpb[:, :, :D] = vs.astype(bf16)
    vpb[:, :, D] = np.float32(1.0)
    vp = (vpb.reshape(B * H, 2 * NPAIR, 128, 128)
          .transpose(0, 2, 1, 3).reshape(B * H, 128, 2 * NPAIR * 128))
    in_maps = []
    for c in range(N_CORES):
        sl = slice(c * HPC, (c + 1) * HPC)
        in_maps.append({
            "QT": np.ascontiguousarray(qt[sl]),
            "KT": np.ascontiguousarray(kt[sl]),
            "VP": np.ascontiguousarray(vp[sl]),
        })
    return in_maps


def kernel(Q, K, V, d_k, **run_kwargs):
    scale = 1.0 / math.sqrt(float(d_k))
    nc = _get_nc(scale)
    in_maps = shard_inputs(Q, K, V)
    res = run_bass_kernel_spmd(nc, in_maps, core_ids=list(range(N_CORES)),
                               **run_kwargs)
    # device output is [heads, 65, S]: rows 0:64 = sum_k p*V transposed,
    # row 64 = softmax denominator. Normalize + transpose on host.
    outs = []
    for r in res.results:
        o = r["out"]                                   # [HPC, 65, S] f32
        outs.append((o[:, :D, :] / o[:, D : D + 1, :]).transpose(0, 2, 1))
    out = np.concatenate(outs, axis=0).reshape(B, H, S, D)
    out = np.ascontiguousarray(out, dtype=np.float32)
    kernel.last_results = res
    return out


# revision 8
# speedup vs baseline: 1.1030x; 1.1030x over previous
"""Multi-head attention on 8 TRN2 NeuronCores (data/head-parallel).

Problem: B=4 H=16 S=2048 D=64 fp32 attention, out = softmax(Q K^T / sqrt(D)) V.
B*H = 64 (batch, head) pairs are sharded 8-per-core; each core runs the same
NEFF over its own 8 heads, no collectives.

v4 design (v1 ~287us, v2 ~257us, v3 ~253us):
  - QK^T uses 2-way PE row tiling (64x128 mode, tiles T0/T8): k-tile 2j's
    K^T sits in array rows 0:64, k-tile 2j+1's in rows 64:128, Q^T is
    host-duplicated into both SBUF partition halves, and the two score
    matmuls run concurrently (~2x QK). One stationary load covers 1024 q
    columns (2 sub-blocks) to amortize the LDWEIGHTS that cannot hide
    behind a same-row-group matmul.
  - exp is spread over THREE engines. ACT does exact exp for most k-tile
    pairs. The rest use a one-instruction Schraudolph exponential —
    i16 = round(score * 128*log2(e)/sqrt(D) + 16248.5) is the bit pattern
    of bf16(exp(score/sqrt(D))) to ~1.8% rms — either directly on DVE
    (reads PSUM), or on GpSimd (cannot touch PSUM) fed by a DVE
    fp32->bf16 bounce copy. Softmax's scale invariance cancels the
    common-mode part of the approximation error.
  - V gets a ones column appended, so PV's PSUM accumulator holds the
    unnormalized output transpose [65, q] with softmax denominators in
    row 64. It is copied to SBUF and DMAd out as-is; division and the
    [d, q] -> [q, d] transpose happen on the host, freeing PE transposes
    and the DVE reciprocal/scale chain.
  - All steady-state DMA issue lives on the Sync queue (GpSimd now does
    exp work); kt/vp are host-packed so every load is contiguous per
    partition (2-4KB descriptors).
"""

import math
from contextlib import ExitStack

import ml_dtypes
import numpy as np

import concourse.bass as bass
import concourse.bacc as bacc
import concourse.tile as tile
import concourse.mybir as mybir
from concourse.bass_utils import run_bass_kernel_spmd

B, H, S, D = 4, 16, 2048, 64
N_CORES = 8
HPC = B * H // N_CORES     # heads per core
NPAIR = 8                  # k-tile pairs (16 k-tiles of 128)
QB = 512                   # q sub-block (one PSUM bank of scores per k-tile)
SB = 1024                  # q super-block (one stationary load per k-tile)
NSB = S // SB
DT = mybir.dt

# Schraudolph-in-bf16-bit-space constants (exp(score/8) ~= bf16 bits of
# round(score * A + Bc) as int16). Bc calibrated for round-to-nearest.
EXP_A = 128.0 * 1.4426950408889634 / 8.0
EXP_B = 16248.5
# exp engine per (pair, sub-block) slot: 'a' = ACT exact, 'v' = DVE
# Schraudolph from PSUM. (A GpSimd path was tried and reverted: GpSimd
# cannot read PSUM and the DVE bounce copy costs DVE more than doing the
# Schraudolph directly.)
ENGINE_MAP = {
    0: "aa", 1: "vv", 2: "aa", 3: "aa",
    4: "vv", 5: "aa", 6: "vv", 7: "aa",
}

_BUILT = {}


class _Bacc(bacc.Bacc):
    """Bacc with the move-matmul-waits-to-ldweights pass disabled: keeping
    waits on the matmul (not its LDWEIGHTS) lets the PE queue pull weight
    loads ahead of in-flight matmuls, hiding the ~70ns LDW cost."""

    def move_matmul_waits_to_ldweights(self):
        pass


def _head(nc, pools, scale, qt_d, kt_d, vp_d, o_d, h):
    (stage, epool, spool, ps_st, ps_ot) = pools

    # ---- loads (bf16, pre-transposed + packed on host) -------------------
    # qt rows 0:64 / 64:128 both hold Q^T. kt packs k-tile pairs: rows 0:64
    # = K^T of tile 2j, rows 64:128 = K^T of tile 2j+1, pair-major in the
    # free dim. vp is host-packed partition-major: vp[p, t*128:] = V' row
    # t*128+p. All three are contiguous per partition in HBM.
    qt = stage.tile([128, S], DT.bfloat16, tag="qt")
    kt = stage.tile([128, NPAIR * 128], DT.bfloat16, tag="kt")
    vp = stage.tile([128, 2 * NPAIR * 128], DT.bfloat16, tag="vp")
    if h == 0:
        # first matmul group needs kt + qt's first super-block; issue those
        # on otherwise-idle queues to cut the cold prologue.
        nc.scalar.dma_start(out=kt, in_=kt_d[h])
        nc.scalar.dma_start(out=qt[:, 0:SB], in_=qt_d[h][:, 0:SB])
        nc.sync.dma_start(out=qt[:, SB:], in_=qt_d[h][:, SB:])
        nc.gpsimd.dma_start(out=vp, in_=vp_d[h])
    else:
        nc.gpsimd.dma_start(out=kt, in_=kt_d[h])
        for j in range(2):
            half = slice(j * (S // 2), (j + 1) * (S // 2))
            nc.gpsimd.dma_start(out=qt[:, half], in_=qt_d[h][:, half])
        nc.gpsimd.dma_start(out=vp, in_=vp_d[h])

    # ---- attention per q super-block -------------------------------------
    for c in range(NSB):
        q0 = c * SB
        ets = []
        for j in range(NPAIR):
            # row-tiled score pair: T0 (rows 0:64) does k-tile 2j, T8
            # (rows 64:128) does k-tile 2j+1, concurrently. One stationary
            # load covers both 512-wide sub-blocks; the sub-blocks land in
            # adjacent PSUM banks of per-sub-block [128, 1024] tiles.
            st0 = ps_st.tile([128, 2 * QB], DT.float32, tag="st")
            st1 = ps_st.tile([128, 2 * QB], DT.float32, tag="st")
            for s, st in enumerate((st0, st1)):
                nc.tensor.matmul(
                    st[:, 0:QB],
                    lhsT=kt[0:64, j * 128 : (j + 1) * 128],
                    rhs=qt[0:64, q0 + s * QB : q0 + (s + 1) * QB],
                    start=True,
                    stop=True,
                )
            for s, st in enumerate((st0, st1)):
                nc.tensor.matmul(
                    st[:, QB : 2 * QB],
                    lhsT=kt[64:128, j * 128 : (j + 1) * 128],
                    rhs=qt[64:128, q0 + s * QB : q0 + (s + 1) * QB],
                    start=True,
                    stop=True,
                )
            for s, st in enumerate((st0, st1)):
                et = epool.tile([128, 2 * QB], DT.bfloat16, tag=f"et{j}_{s}")
                eng = ENGINE_MAP[j][s]
                if eng == "a":
                    nc.scalar.activation(
                        out=et, in_=st,
                        func=mybir.ActivationFunctionType.Exp, scale=scale,
                    )
                else:
                    nc.vector.tensor_scalar(
                        et.bitcast(DT.int16), st, EXP_A, EXP_B,
                        mybir.AluOpType.mult, mybir.AluOpType.add,
                    )
                ets.append(et)

        ot = ps_ot.tile([128, SB], DT.float32, tag="ot")
        for s in range(2):
            for j in range(NPAIR):
                for half in range(2):
                    t = 2 * j + half
                    nc.tensor.matmul(
                        ot[:, s * QB : (s + 1) * QB],
                        lhsT=vp[:, t * 128 : (t + 1) * 128],
                        rhs=ets[2 * j + s][:, half * QB : (half + 1) * QB],
                        start=(t == 0),
                        stop=(t == 2 * NPAIR - 1),
                    )
        # unnormalized out^T: rows 0:64 numerator, row 64 denominator.
        # Division + transpose happen on the host.
        ots = spool.tile([D + 1, SB], DT.float32, tag="ots")
        nc.vector.tensor_copy(out=ots, in_=ot[0 : D + 1, :])
        nc.sync.dma_start(out=o_d[h][:, q0 : q0 + SB], in_=ots)


def build_graph(scale: float, heads: int = HPC):
    nc = _Bacc("TRN2", target_bir_lowering=False, debug=False,
               num_devices=N_CORES)
    qt_d = nc.dram_tensor("QT", [heads, 128, S], DT.bfloat16,
                          kind="ExternalInput").ap()
    kt_d = nc.dram_tensor("KT", [heads, 128, NPAIR * 128], DT.bfloat16,
                          kind="ExternalInput").ap()
    vp_d = nc.dram_tensor("VP", [heads, 128, 2 * NPAIR * 128], DT.bfloat16,
                          kind="ExternalInput").ap()
    o_d = nc.dram_tensor("out", [heads, D + 1, S], DT.float32,
                         kind="ExternalOutput").ap()

    with tile.TileContext(nc) as tc, ExitStack() as ctx:
        stage = ctx.enter_context(tc.tile_pool(name="stage", bufs=2))
        epool = ctx.enter_context(tc.tile_pool(name="epool", bufs=2))
        spool = ctx.enter_context(tc.tile_pool(name="spool", bufs=3))
        ps_st = ctx.enter_context(tc.tile_pool(name="ps_st", bufs=3, space="PSUM"))
        ps_ot = ctx.enter_context(tc.tile_pool(name="ps_ot", bufs=1, space="PSUM"))

        pools = (stage, epool, spool, ps_st, ps_ot)
        for h in range(heads):
            _head(nc, pools, scale, qt_d, kt_d, vp_d, o_d, h)

    nc.compile()
    return nc


def _get_nc(scale: float):
    key = round(float(scale), 9)
    if key not in _BUILT:
        _BUILT[key] = build_graph(float(scale))
    return _BUILT[key]


def shard_inputs(Q, K, V):
    """Host-side prep: shard heads across cores; build qt (Q^T duplicated
    into both 64-row halves), kt (k-tile pairs packed for row tiling), vp
    (V plus a ones column, partition-major so DMA is contiguous)."""
    bf16 = ml_dtypes.bfloat16
    qs = np.asarray(Q, dtype=np.float32).reshape(B * H, S, D)
    ks = np.asarray(K, dtype=np.float32).reshape(B * H, S, D)
    vs = np.asarray(V, dtype=np.float32).reshape(B * H, S, D)
    qtT = qs.transpose(0, 2, 1).astype(bf16)          # [BH, D, S]
    qt = np.concatenate([qtT, qtT], axis=1)           # [BH, 128, S]
    ktT = ks.transpose(0, 2, 1).astype(bf16)          # [BH, D, S]
    # pairs: rows 0:64 <- k-tile 2j, rows 64:128 <- k-tile 2j+1
    ktv = ktT.reshape(B * H, D, NPAIR, 2, 128)
    kt = np.empty((B * H, 128, NPAIR, 128), dtype=bf16)
    kt[:, :D] = ktv[:, :, :, 0, :]
    kt[:, D:] = ktv[:, :, :, 1, :]
    kt = kt.reshape(B * H, 128, NPAIR * 128)
    # vp partition-major: vp[bh, p, t*128 + e] = V'[bh, t*128 + p, e]
    vpb = np.zeros((B * H, S, 128), dtype=bf16)
    vpb[:, :, :D] = vs.astype(bf16)
    vpb[:, :, D] = np.float32(1.0)
    vp = (vpb.reshape(B * H, 2 * NPAIR, 128, 128)
          .transpose(0, 2, 1, 3).reshape(B * H, 128, 2 * NPAIR * 128))
    in_maps = []
    for c in range(N_CORES):
        sl = slice(c * HPC, (c + 1) * HPC)
        in_maps.append({
            "QT": np.ascontiguousarray(qt[sl]),
            "KT": np.ascontiguousarray(kt[sl]),
            "VP": np.ascontiguousarray(vp[sl]),
        })
    return in_maps


def kernel(Q, K, V, d_k, **run_kwargs):
    scale = 1.0 / math.sqrt(float(d_k))
    nc = _get_nc(scale)
    in_maps = shard_inputs(Q, K, V)
    res = run_bass_kernel_spmd(nc, in_maps, core_ids=list(range(N_CORES)),
                               **run_kwargs)
    # device output is [heads, 65, S]: rows 0:64 = sum_k p*V transposed,
    # row 64 = softmax denominator. Normalize + transpose on host.
    outs = []
    for r in res.results:
        o = r["out"]                                   # [HPC, 65, S] f32
        outs.append((o[:, :D, :] / o[:, D : D + 1, :]).transpose(0, 2, 1))
    out = np.concatenate(outs, axis=0).reshape(B, H, S, D)
    out = np.ascontiguousarray(out, dtype=np.float32)
    kernel.last_results = res
    return out
